# revision 2
# baseline (speedup 1.0000x reference)
"""Multi-head self-attention (B=4, S=1024, D=512, H=8) on 8 trn2 NeuronCores.

Sharding: tensor-parallel over heads -- core h computes head h end-to-end.

Per-core math (head h), exploiting softmax row-shift invariance and
attn-rows-sum-to-1 to fold the four projection matmuls into two:
    A = Wq_h @ Wk_h^T / sqrt(D)          (host, fp32 -> fp16)
    v = bq_h @ Wk_h^T / sqrt(D)          (host)
    C = Wv_h @ Wo_h                      (host)
    G^T = A^T x^T + v                    (device)   [d, tok]
    scoresT = x G^T ,  scores = G x^T    (device, both orientations so the
                                          softmax reduce and the ctx matmul
                                          both have their contraction on
                                          partitions -- no transposes)
    E = exp(scores / sqrt(D))            (no max-subtract: |logits| < ~0.3)
    attn = E / rowsum(E)                 -> output [B,S,S] fp32
    out_p = (E^T-matmul with U) * 1/rowsum,  U = x C   -> partial [TOK, D]
Host: out = sum_h out_p_h + (bv @ Wo + bo), attn stacked head-major.
"""

import os

os.environ.setdefault("MYCRO_LOCAL_CACHE", "1")

import numpy as np

B, S, D, H = 4, 1024, 512, 8
TOK = B * S  # 4096
N_CORES = 8
P = 128
KT = D // P  # 4 k-tiles of 128 over the d dimension
SCL = 1.0 / float(np.sqrt(D))  # second half of the 1/D score scale

_cache = {}


def build_program():
    """Build the single-core Bass/Tile program (SPMD across 8 cores)."""
    import concourse.tile as tile
    from concourse import bacc, mybir

    f16 = mybir.dt.float16
    f32 = mybir.dt.float32
    AFT = mybir.ActivationFunctionType
    AX = mybir.AxisListType

    nc = bacc.Bacc("TRN2", debug=False, num_devices=N_CORES)

    xT = nc.dram_tensor("xT", [D, TOK], f16, kind="ExternalInput").ap()
    Am = nc.dram_tensor("Am", [D, D], f16, kind="ExternalInput").ap()
    Cm = nc.dram_tensor("Cm", [D, D], f16, kind="ExternalInput").ap()
    vb = nc.dram_tensor("vb", [P, KT], f32, kind="ExternalInput").ap()
    attn_o = nc.dram_tensor("attn_o", [B, S, S], f32, kind="ExternalOutput").ap()
    out_p = nc.dram_tensor("out_p", [TOK, D], f32, kind="ExternalOutput").ap()

    xT_t = xT.rearrange("(po pi) t -> pi po t", pi=P)
    Am_t = Am.rearrange("(po pi) d -> pi po d", pi=P)
    Cm_t = Cm.rearrange("(po pi) d -> pi po d", pi=P)

    with tile.TileContext(nc) as tc:
        with (
            tc.tile_pool(name="persist", bufs=1) as persist,
            tc.tile_pool(name="etp", bufs=2) as etp,
            tc.tile_pool(name="recp", bufs=2) as recp,
            tc.tile_pool(name="eijp", bufs=3) as eijp,
            tc.tile_pool(name="attnp", bufs=3) as attnp,
            tc.tile_pool(name="opp", bufs=3) as opp,
            tc.tile_pool(name="smallp", bufs=8) as smallp,
            tc.tile_pool(name="psp", bufs=6, space="PSUM") as psp,
        ):
            # ---- persistent SBUF state -------------------------------------
            xT_sb = persist.tile([P, KT, TOK], f16)
            for po in range(KT):
                nc.sync.dma_start(xT_sb[:, po, :], xT_t[:, po, :])
            A_sb = persist.tile([P, KT, D], f16)
            nc.sync.dma_start(A_sb[:], Am_t)
            C_sb = persist.tile([P, KT, D], f16)
            nc.sync.dma_start(C_sb[:], Cm_t)
            vb_sb = persist.tile([P, KT], f32)
            nc.sync.dma_start(vb_sb[:], vb[:])

            GT_sb = persist.tile([P, KT, TOK], f16)
            U_sb = persist.tile([P, TOK // P, D], f16)

            # ---- stage 1: G^T[d_out, tok] = A^T x^T + v --------------------
            for po in range(KT):
                for ch in range(TOK // 512):
                    ps = psp.tile([P, 512], f32)
                    for k in range(KT):
                        nc.tensor.matmul(
                            ps[:],
                            A_sb[:, k, po * P : (po + 1) * P],
                            xT_sb[:, k, ch * 512 : (ch + 1) * 512],
                            start=(k == 0),
                            stop=(k == KT - 1),
                        )
                    nc.scalar.activation(
                        GT_sb[:, po, ch * 512 : (ch + 1) * 512],
                        ps[:],
                        AFT.Identity,
                        bias=vb_sb[:, po : po + 1],
                        scale=1.0,
                    )

            # ---- stage 2: U[tok, d_out] = x C ------------------------------
            for jt in range(TOK // P):
                ps = psp.tile([P, 512], f32)
                for k in range(KT):
                    nc.tensor.matmul(
                        ps[:],
                        xT_sb[:, k, jt * P : (jt + 1) * P],
                        C_sb[:, k, :],
                        start=(k == 0),
                        stop=(k == KT - 1),
                    )
                nc.scalar.copy(U_sb[:, jt, :], ps[:])

            # ---- stage 3: per-batch attention ------------------------------
            for b in range(B):
                t0 = b * S  # batch token offset

                # 3a: ET[j, i] = exp(scores[i, j] * SCL), keys on partitions
                ET_sb = etp.tile([P, S // P, S], f16)
                for jt in range(S // P):
                    for ic in range(S // 512):
                        ps = psp.tile([P, 512], f32)
                        for k in range(KT):
                            nc.tensor.matmul(
                                ps[:],
                                xT_sb[:, k, t0 + jt * P : t0 + (jt + 1) * P],
                                GT_sb[:, k, t0 + ic * 512 : t0 + (ic + 1) * 512],
                                start=(k == 0),
                                stop=(k == KT - 1),
                            )
                        nc.scalar.activation(
                            ET_sb[:, jt, ic * 512 : (ic + 1) * 512],
                            ps[:],
                            AFT.Exp,
                            scale=SCL,
                        )

                # 3b: E[i, j] + rowsum + normalize -> attn output
                rec_sb = recp.tile([P, S // P], f32)
                for it in range(S // P):
                    eij = eijp.tile([P, S], f32)
                    rs2 = smallp.tile([P, 2], f32)
                    for jc in range(S // 512):
                        ps = psp.tile([P, 512], f32)
                        for k in range(KT):
                            nc.tensor.matmul(
                                ps[:],
                                GT_sb[:, k, t0 + it * P : t0 + (it + 1) * P],
                                xT_sb[:, k, t0 + jc * 512 : t0 + (jc + 1) * 512],
                                start=(k == 0),
                                stop=(k == KT - 1),
                            )
                        nc.scalar.activation(
                            eij[:, jc * 512 : (jc + 1) * 512],
                            ps[:],
                            AFT.Exp,
                            scale=SCL,
                            accum_out=rs2[:, jc : jc + 1],
                        )
                    rs1 = smallp.tile([P, 1], f32)
                    nc.vector.reduce_sum(rs1[:], rs2[:], axis=AX.X)
                    nc.vector.reciprocal(rec_sb[:, it : it + 1], rs1[:])
                    attn_sb = attnp.tile([P, S], f32)
                    nc.vector.tensor_scalar_mul(
                        attn_sb[:], eij[:], rec_sb[:, it : it + 1]
                    )
                    nc.sync.dma_start(
                        attn_o[b, it * P : (it + 1) * P, :], attn_sb[:]
                    )

                # 3c: out_p[i, d] = (sum_j ET[j,i] U[j,d]) * rec[i]
                for it in range(S // P):
                    ps = psp.tile([P, 512], f32)
                    for jt in range(S // P):
                        nc.tensor.matmul(
                            ps[:],
                            ET_sb[:, jt, it * P : (it + 1) * P],
                            U_sb[:, b * (S // P) + jt, :],
                            start=(jt == 0),
                            stop=(jt == S // P - 1),
                        )
                    op_sb = opp.tile([P, D], f32)
                    nc.vector.tensor_scalar_mul(op_sb[:], ps[:], rec_sb[:, it : it + 1])
                    nc.sync.dma_start(
                        out_p[(b * (S // P) + it) * P : (b * (S // P) + it + 1) * P, :],
                        op_sb[:],
                    )

    nc.compile()
    return nc


def make_in_maps(x, Wq, bq, Wk, bk, Wv, bv, Wo, bo):
    """Host-side prep: transpose x, fold weights per head, build per-core inputs."""
    x = np.asarray(x, dtype=np.float32)
    Wq = np.asarray(Wq, dtype=np.float32)
    Wk = np.asarray(Wk, dtype=np.float32)
    Wv = np.asarray(Wv, dtype=np.float32)
    Wo = np.asarray(Wo, dtype=np.float32)
    bq = np.asarray(bq, dtype=np.float32)
    bv = np.asarray(bv, dtype=np.float32)
    bo = np.asarray(bo, dtype=np.float32)

    xT16 = np.ascontiguousarray(x.reshape(TOK, D).T).astype(np.float16)
    sq = np.float32(np.sqrt(D))

    in_maps = []
    for h in range(H):
        sl = slice(h * D, (h + 1) * D)
        Wq_h, Wk_h, Wv_h, Wo_h = Wq[:, sl], Wk[:, sl], Wv[:, sl], Wo[sl, :]
        A_h = (Wq_h @ Wk_h.T) / sq
        v_h = (bq[sl] @ Wk_h.T) / sq
        C_h = Wv_h @ Wo_h
        in_maps.append(
            {
                "xT": xT16,
                "Am": np.ascontiguousarray(A_h).astype(np.float16),
                "Cm": np.ascontiguousarray(C_h).astype(np.float16),
                "vb": np.ascontiguousarray(v_h.reshape(KT, P).T).astype(np.float32),
            }
        )
    bias_vec = bv @ Wo + bo  # == sum_h bv_h @ Wo_h + bo
    return in_maps, bias_vec


LAST_RESULT = None


def kernel(x, Wq, bq, Wk, bk, Wv, bv, Wo, bo):
    global LAST_RESULT
    from concourse.bass_utils import run_bass_kernel_spmd

    if "nc" not in _cache:
        _cache["nc"] = build_program()
    nc = _cache["nc"]

    in_maps, bias_vec = make_in_maps(x, Wq, bq, Wk, bk, Wv, bv, Wo, bo)

    res = run_bass_kernel_spmd(nc, in_maps, list(range(N_CORES)))
    LAST_RESULT = res

    attn_full = np.empty((H * B, S, S), dtype=np.float32)
    out_acc = np.zeros((TOK, D), dtype=np.float32)
    for h in range(H):
        attn_full[h * B : (h + 1) * B] = res.results[h]["attn_o"]
        out_acc += res.results[h]["out_p"]
    out_full = (out_acc + bias_vec[None, :]).reshape(B, S, D).astype(np.float32)
    return out_full, attn_full
